# revision 28
# baseline (speedup 1.0000x reference)
"""Trainium2 Bass kernel for multi-head GQA attention (dense transformer layer).

Problem: x[2,2048,4096] -> attention(RoPE, GQA 32q/8kv heads, causal) -> out[2,2048,4096]

Strategy (8 NeuronCores):
  - QKV tensor-parallel by heads: core c owns q-heads 4c..4c+3 and kv-head c.
    Activations kept feature-on-partition: X^T [din, tok], Q^T/K^T [d, tok],
    scores^T [k, q] -- softmax denominators are ones-matmuls, no transposes.
  - Attention outputs are resharded head->token with a per-batch AllToAll
    (2 MB per core instead of a 16 MB-out AllGather); each core then computes
    the output projection for its own 256 tokens per batch against the FULL
    wo matrix, streamed from DRAM (weights have no reuse there anyway).
  - gpsimd queue carries ONLY the collectives; the softmax-denominator
    broadcast is a tiny ones-matmul on PE, so the AllToAll never blocks the
    attention pipeline of the next batch.
  - Matmuls in bf16 with fp32 PSUM accumulation (optionally fp8 DoubleRow for
    the projections; layouts are [128, kk, 2, m] so both paths share them).
"""

import numpy as np
from contextlib import ExitStack

import concourse.bass as bass
import concourse.tile as tile
from concourse import bacc, mybir
from concourse.bass import ts
from concourse.bass_utils import run_bass_kernel_spmd

BF16 = mybir.dt.bfloat16
F32 = mybir.dt.float32
FP8 = mybir.dt.float8e4

N_CORES = 8
DIM = 4096
N_HEADS = 32
HEAD_DIM = 128
BATCH = 2
SEQ = 2048

TOK = BATCH * SEQ            # 4096 tokens, batch-major
NB = TOK // 512              # 8 token blocks of 512
KT2 = DIM // 256             # 16 double-contraction tiles for projections
H_PER_CORE = N_HEADS // N_CORES       # 4
DQ = H_PER_CORE * HEAD_DIM            # 512 q-dims per core
QB = SEQ // 512              # 4 query blocks of 512 per batch
SKT = SEQ // 128             # 16 key tiles of 128 per batch
TPC = SEQ // N_CORES         # 256 tokens owned per core per batch

# ---- dtype configuration (fp8 flips these) ----
PROJ_FP8 = False             # QKV projection inputs/weights in fp8 e4m3
WO_FP8 = False               # wo weights + attention outputs in fp8 e4m3
PDT = FP8 if PROJ_FP8 else BF16
WDT = FP8 if WO_FP8 else BF16
XSI = 16.0 if PROJ_FP8 else 1.0      # x pre-scale (host)
WSI = 256.0 if PROJ_FP8 else 1.0     # wq/wk/wv pre-scale (host)
PF = XSI * WSI                        # QKV psum excess factor
ATSI = 16.0 if WO_FP8 else 1.0       # attention-output pre-scale (device)
WOSI = 256.0 if WO_FP8 else 1.0      # wo pre-scale (host)
OSC = 1.0 / (ATSI * WOSI)            # final output correction

DR = mybir.MatmulPerfMode.DoubleRow

EXP = mybir.ActivationFunctionType.Exp


def build_program(reps: int = 1, n_cores: int = N_CORES,
                  collective: bool = True, phases: str = "full") -> bass.Bass:
    nc = bacc.Bacc("TRN2", target_bir_lowering=False, debug=False,
                   num_devices=n_cores)

    # ---- I/O (per-core tensors; host pre-arranges layouts) ----
    xT = nc.dram_tensor("xT", [KT2, NB, 128, 2, 512], PDT, kind="ExternalInput").ap()
    wq = nc.dram_tensor("wq", [128, KT2 * 2 * DQ], PDT, kind="ExternalInput").ap()
    wk = nc.dram_tensor("wk", [128, KT2 * 2 * 128], PDT, kind="ExternalInput").ap()
    wv = nc.dram_tensor("wv", [128, KT2 * 2 * 128], PDT, kind="ExternalInput").ap()
    wos = nc.dram_tensor("wos", [32, 128, KT2 * 2 * 128], WDT,
                         kind="ExternalInput").ap()
    cosT = nc.dram_tensor("cosT", [128, SEQ], BF16, kind="ExternalInput").ap()
    sinT = nc.dram_tensor("sinT", [128, SEQ], BF16, kind="ExternalInput").ap()
    pmat = nc.dram_tensor("pmat", [128, 128], BF16, kind="ExternalInput").ap()
    tri = nc.dram_tensor("tri", [128, 128], BF16, kind="ExternalInput").ap()
    ident = nc.dram_tensor("ident", [128, 128], BF16, kind="ExternalInput").ap()
    ones = nc.dram_tensor("ones", [128, 128], BF16, kind="ExternalInput").ap()
    outT = nc.dram_tensor("outT", [DIM, BATCH * TPC], BF16, kind="ExternalOutput").ap()

    # internal DRAM for the collectives (cannot use I/O tensors); one pair
    # per (rep, batch) so reps never race and batches can overlap.
    cc_in = [[nc.dram_tensor(f"cc_in{r}_{b}", [N_CORES, DQ, TPC], WDT)
              for b in range(BATCH)] for r in range(reps)]
    cc_out = [[nc.dram_tensor(f"cc_out{r}_{b}", [N_CORES, DQ, TPC], WDT)
               for b in range(BATCH)] for r in range(reps)]

    with tile.TileContext(nc) as tc, ExitStack() as top:
        consts = top.enter_context(tc.tile_pool(name="consts", bufs=1))
        weights = top.enter_context(tc.tile_pool(name="weights", bufs=1))
        acts = top.enter_context(tc.tile_pool(name="acts", bufs=1))

        # weights: the very first matmuls gate only on the kk=0..3 chunks;
        # everything else is emitted right after the first x-tile DMA (see
        # the late_loads callback below).
        wq_sb = weights.tile([128, KT2, 2, DQ], PDT)
        wk_sb = weights.tile([128, KT2, 2, 128], PDT)
        wv_sb = weights.tile([128, KT2, 2, 128], PDT)
        wq4 = wq.rearrange("p (k i m) -> p k i m", k=KT2, i=2)
        wk4 = wk.rearrange("p (k i m) -> p k i m", k=KT2, i=2)
        wv4 = wv.rearrange("p (k i m) -> p k i m", k=KT2, i=2)
        ck = 2
        for kki in range(ck):
            k0 = slice(kki, kki + 1)
            nc.sync.dma_start(wq_sb[:, k0], wq4[:, k0])
            nc.sync.dma_start(wk_sb[:, k0], wk4[:, k0])
            nc.sync.dma_start(wv_sb[:, k0], wv4[:, k0])

        # tiny constants next
        pm_sb = consts.tile([128, 128], BF16)
        nc.sync.dma_start(pm_sb[:], pmat[:, :])
        tri_sb = consts.tile([128, 128], BF16)
        nc.sync.dma_start(tri_sb[:], tri[:, :])
        id_sb = consts.tile([128, 128], BF16)
        nc.sync.dma_start(id_sb[:], ident[:, :])
        ones_sb = consts.tile([128, 128], BF16)
        nc.sync.dma_start(ones_sb[:], ones[:, :])
        atsi_sb = consts.tile([1, 128], BF16)
        nc.vector.memset(atsi_sb[:], ATSI)

        cos_sb = consts.tile([128, SEQ], BF16)
        sin_sb = consts.tile([128, SEQ], BF16)

        def late_loads():
            ksl = slice(ck, KT2)  # ck == 2
            nc.sync.dma_start(wq_sb[:, ksl], wq4[:, ksl])
            nc.sync.dma_start(wk_sb[:, ksl], wk4[:, ksl])
            nc.sync.dma_start(wv_sb[:, ksl], wv4[:, ksl])
            nc.sync.dma_start(cos_sb[:], cosT[:, :])
            nc.sync.dma_start(sin_sb[:], sinT[:, :])
        run_body.late_loads = late_loads

        # per-core activations (feature-major)
        qt_sb = [acts.tile([128, TOK], BF16, tag=f"qt{m}", name=f"qt{m}")
                 for m in range(H_PER_CORE)]
        kt_sb = acts.tile([128, TOK], BF16)
        vt_sb = acts.tile([128, TOK], BF16)          # V^T, pre-transpose
        va_sb = acts.tile([128, 2 * KT2, 128], BF16)  # V in [tok, dv] tiles

        run_body.wos_dram = wos
        for rep in range(reps):
            run_body(nc, tc, rep, cc_in[rep], cc_out[rep], outT,
                     wq_sb, wk_sb, wv_sb, cos_sb, sin_sb, pm_sb,
                     tri_sb, id_sb, ones_sb, atsi_sb, qt_sb, kt_sb, vt_sb,
                     va_sb, xT, wos, n_cores=n_cores, collective=collective,
                     phases=phases)

    nc.compile()
    return nc


def run_body(nc, tc, rep, cc_in, cc_out, outT,
             wq_sb, wk_sb, wv_sb, cos_sb, sin_sb, pm_sb,
             tri_sb, id_sb, ones_sb, atsi_sb, qt_sb, kt_sb, vt_sb, va_sb, xT,
             wos, n_cores=N_CORES, collective=True, phases="full"):
    # ---------------- phase 1: QKV projection + RoPE ----------------
    with ExitStack() as body:
      ps = body.enter_context(tc.tile_pool(name=f"ps_{rep}", bufs=1, space="PSUM"))
      with ExitStack() as ph:
        xin = ph.enter_context(tc.tile_pool(name=f"xin{rep}", bufs=4))
        rope = ph.enter_context(tc.tile_pool(name=f"rope{rep}", bufs=2))

        for n in range(NB):
            s0 = n % QB                 # 512-block position within the batch
            q_ps = [ps.tile([128, 512], F32, tag=f"qps{m}", bufs=1, name=f"qps{m}")
                    for m in range(H_PER_CORE)]
            k_ps = ps.tile([128, 512], F32, tag="kps", bufs=1)
            v_ps = ps.tile([128, 512], F32, tag="vps", bufs=1)
            for kk in range(KT2):
                xt = xin.tile([128, 2, 512], PDT, tag="xt")
                nc.sync.dma_start(xt[:], xT[kk, n])
                if rep == 0 and n == 0 and kk == 2 and run_body.late_loads:
                    run_body.late_loads()
                    run_body.late_loads = None
                st, sp = (kk == 0), (kk == KT2 - 1)
                if PROJ_FP8:
                    for m in range(H_PER_CORE):
                        nc.tensor.matmul(q_ps[m][:], wq_sb[:, kk, :, ts(m, 128)],
                                         xt[:], start=st, stop=sp, perf_mode=DR)
                    nc.tensor.matmul(k_ps[:], wk_sb[:, kk], xt[:],
                                     start=st, stop=sp, perf_mode=DR)
                    nc.tensor.matmul(v_ps[:], wv_sb[:, kk], xt[:],
                                     start=st, stop=sp, perf_mode=DR)
                else:
                    for i in range(2):
                        sti, spi = st and i == 0, sp and i == 1
                        for m in range(H_PER_CORE):
                            nc.tensor.matmul(q_ps[m][:], wq_sb[:, kk, i, ts(m, 128)],
                                             xt[:, i, :], start=sti, stop=spi)
                        nc.tensor.matmul(k_ps[:], wk_sb[:, kk, i], xt[:, i, :],
                                         start=sti, stop=spi)
                        nc.tensor.matmul(v_ps[:], wv_sb[:, kk, i], xt[:, i, :],
                                         start=sti, stop=spi)

            # V^T: copy out of PSUM (scale folds away the fp8 pre-scales)
            if PF == 1.0:
                nc.scalar.copy(vt_sb[:, ts(n, 512)], v_ps[:])
            else:
                nc.scalar.mul(vt_sb[:, ts(n, 512)], v_ps[:], 1.0 / PF)

            # RoPE on Q heads and K:  y = raw*cos + (P@raw)*sin
            # (softmax scale and 1/PF folded into the cos/sin tables)
            def do_rope(acc, dst, eng):
                raw = rope.tile([128, 512], BF16, tag="raw")
                eng.tensor_copy(raw[:], acc[:]) if eng is nc.vector \
                    else eng.copy(raw[:], acc[:])
                rot = ps.tile([128, 512], F32, tag="rot", bufs=2)
                nc.tensor.matmul(rot[:], pm_sb[:], raw[:], start=True, stop=True)
                t1 = rope.tile([128, 512], BF16, tag="t1")
                nc.vector.tensor_mul(t1[:], raw[:], cos_sb[:, ts(s0, 512)])
                t2 = rope.tile([128, 512], BF16, tag="t2")
                nc.vector.tensor_mul(t2[:], rot[:], sin_sb[:, ts(s0, 512)])
                nc.vector.tensor_add(dst, t1[:], t2[:])

            if phases == "norope":
                for m in range(H_PER_CORE):
                    eng = nc.scalar if m % 2 == 0 else nc.vector
                    if eng is nc.vector:
                        eng.tensor_copy(qt_sb[m][:, ts(n, 512)], q_ps[m][:])
                    else:
                        eng.copy(qt_sb[m][:, ts(n, 512)], q_ps[m][:])
                nc.scalar.copy(kt_sb[:, ts(n, 512)], k_ps[:])
            else:
                for m in range(H_PER_CORE):
                    do_rope(q_ps[m], qt_sb[m][:, ts(n, 512)],
                            nc.scalar if m % 2 == 0 else nc.vector)
                do_rope(k_ps, kt_sb[:, ts(n, 512)], nc.scalar)

            # V^T -> V transposes for this block (shares the rot psum slots)
            for t in range(4 * n, 4 * n + 4):
                tr = ps.tile([128, 128], BF16, tag="rot", bufs=2, name="tr")
                nc.tensor.transpose(tr[:], vt_sb[:, ts(t, 128)], id_sb[:])
                nc.vector.tensor_copy(va_sb[:, t, :], tr[:])

      # ------- phase 2: attention; a2a per batch overlaps next phase -------
      with ExitStack() as ph:
        work = ph.enter_context(tc.tile_pool(name=f"attnwork{rep}", bufs=4))

        def attention_batch(b):
            for qb in range(QB):
                gq = b * SEQ + qb * 512
                nkt = (qb + 1) * 4
                atags = ["qps3", "kps", "vps", "rot"]
                o_ps = [ps.tile([128, 512], F32, tag=atags[h],
                                bufs=(2 if atags[h] == "rot" else 1),
                                name=f"aops{h}") for h in range(H_PER_CORE)]
                # one psum bank holds all 4 heads' softmax denominators at
                # 32-aligned partition offsets (col-group packed matmuls)
                dn_ps = ps.tile([128, 512], F32, tag="rot", bufs=2, name="dn")
                ex_prev = [None] * H_PER_CORE
                off_prev = [0] * H_PER_CORE
                for kt in range(nkt):
                    gk = b * SEQ + kt * 128
                    vtile = b * SKT + kt
                    j = kt - qb * 4          # >= 0 -> diagonal tile
                    q_off = 128 * j if j > 0 else 0
                    N = 512 - q_off
                    st, sp = (kt == 0), (kt == nkt - 1)
                    # score matmuls run ahead of the exp+PV loop: the PE works
                    # through the scores while the scalar engine exps the
                    # previous heads, instead of stalling per head. Only 3
                    # psum banks rotate, so head 3's score matmul must be
                    # emitted after head 0's exp (which frees bank 0).
                    s_ps, exs = [], []

                    def emit_s(h):
                        sp_t = ps.tile([128, 512], F32,
                                       tag=f"qps{(kt * 4 + h) % 3}", bufs=1,
                                       name="sps")
                        nc.tensor.matmul(sp_t[:, :N], kt_sb[:, gk:gk + 128],
                                         qt_sb[h][:, gq + q_off:gq + 512],
                                         start=True, stop=True)
                        s_ps.append(sp_t)

                    def emit_exp(h):
                        ex = work.tile([128, 512], BF16, tag="expT", bufs=12,
                                       name="ex")
                        nc.scalar.activation(ex[:, :N], s_ps[h][:, :N], EXP)
                        exs.append(ex)

                    for h in range(3):
                        emit_s(h)
                    emit_exp(0)
                    emit_s(3)
                    for h in range(1, 4):
                        emit_exp(h)
                    for h in range(H_PER_CORE):
                        ex = exs[h]
                        if j >= 0:
                            nc.vector.tensor_mul(ex[:, :128], ex[:, :128],
                                                 tri_sb[:])
                        nc.tensor.matmul(o_ps[h][:, q_off:], va_sb[:, vtile, :],
                                         ex[:, :N], start=st, stop=sp)
                        if kt % 2 == 0:
                            ex_prev[h] = ex
                            off_prev[h] = q_off
                        else:
                            # fold this tile's exp sums into the previous
                            # tile (bf16 add), one denominator matmul per pair
                            exp_, offp = ex_prev[h], off_prev[h]
                            d = q_off - offp
                            nc.vector.tensor_add(exp_[:, d:512 - offp],
                                                 exp_[:, d:512 - offp],
                                                 ex[:, :N])
                            nc.tensor.matmul(
                                dn_ps[32 * h:32 * h + 1, offp:],
                                ones_sb[:, 0:1], exp_[:, :512 - offp],
                                start=(kt == 1), stop=(kt == nkt - 1),
                                tile_position=(0, 32 * h))
                            ex_prev[h] = None
                # normalization chain, grouped per step so the psum "rot"
                # buffers reused by bc are provably drained (all dn/o readers
                # emitted first) and PE never WARs onto the lagging DVE chain.
                recs, rec16s = [], []
                for h in range(H_PER_CORE):
                    rec = work.tile([1, 512], F32, tag="rec", name="rec")
                    nc.vector.reciprocal(rec[:], dn_ps[32 * h:32 * h + 1, :])
                    recs.append(rec)
                for h in range(H_PER_CORE):
                    rec16 = work.tile([1, 512], BF16, tag="rec16", name="rec16")
                    nc.vector.tensor_copy(rec16[:], recs[h][:])
                    rec16s.append(rec16)
                ocs = []
                for h in range(H_PER_CORE):
                    oc = work.tile([128, 512], F32, tag="oc", name="oc")
                    nc.vector.tensor_copy(oc[:], o_ps[h][:])   # frees psum bank
                    ocs.append(oc)
                for h in range(H_PER_CORE):
                    # broadcast ATSI/denominator across partitions via PE
                    bc = ps.tile([128, 512], F32, tag="rot", bufs=2, name="bc")
                    nc.tensor.matmul(bc[:], atsi_sb[:], rec16s[h][:],
                                     start=True, stop=True)
                    at = work.tile([128, 512], WDT, tag="at", name="at")
                    nc.vector.tensor_mul(at[:], ocs[h][:], bc[:])
                    # scatter the two 256-token halves to their a2a chunks
                    # (two partition-major DMAs; a fused chunk-leading AP is
                    # not a legal SBUF access pattern)
                    for i in range(2):
                        nc.sync.dma_start(
                            cc_in[b].ap()[2 * qb + i,
                                          h * 128:(h + 1) * 128, :],
                            at[:, i * TPC:(i + 1) * TPC])

        def a2a_batch(b):
            if collective:
                nc.gpsimd.collective_compute(
                    "AllToAll",
                    mybir.AluOpType.bypass,
                    ins=[cc_in[b].ap().opt()],
                    outs=[cc_out[b].ap().opt()],
                    replica_groups=[list(range(n_cores))],
                )

        ain = ph.enter_context(tc.tile_pool(name=f"ain{rep}", bufs=2))
        wos3 = wos  # [32, 128, KT2*2*128]
        at8 = []

        def at8_loads(b):
            # this core owns tokens [b, TPC*core .. TPC*core+TPC); the a2a
            # output is exactly attn^T [4096 hd, TPC] for those tokens.
            # at8 loads are triggered from the gpsimd queue, IN ORDER AFTER
            # the AllToAll: their descriptors only reach the DMA engines once
            # the collective is done, so they never park at a queue head
            # blocking unrelated transfers. Batch b fills columns
            # [b*TPC, b*TPC+TPC) so the wo matmuls sweep both batches at once.
            at_flat = cc_out[b].ap().rearrange("c r t -> (c r) t")
            for kk in range(KT2):
                if b == 0:
                    at8.append(ain.tile([128, 2, 2 * TPC], WDT,
                                        tag=f"at8_{kk}", bufs=1,
                                        name=f"at8_{kk}"))
                t8 = at8[kk]
                for i in range(2):
                    nc.gpsimd.dma_start(t8[:, i, ts(b, TPC)],
                                        at_flat[256 * kk + 128 * i:
                                                256 * kk + 128 * i + 128, :])

        def wo_all():
            wtags = ["qps3", "kps", "vps", "rot"]
            for m in range(32):
                wt = ain.tile([128, KT2, 2, 128], WDT, tag="wot", bufs=3,
                              name="wot")
                nc.sync.dma_start(wt[:], wos3[m].rearrange(
                    "p (k i m) -> p k i m", k=KT2, i=2))
                o_m = ps.tile([128, 512], F32, tag=wtags[m % 4],
                              bufs=(2 if wtags[m % 4] == "rot" else 1),
                              name=f"wops{m % 4}")
                if WO_FP8:
                    for kk in range(KT2):
                        nc.tensor.matmul(o_m[:], wt[:, kk], at8[kk][:],
                                         start=(kk == 0), stop=(kk == KT2 - 1),
                                         perf_mode=DR)
                else:
                    for kk in range(KT2):
                        for i in range(2):
                            nc.tensor.matmul(
                                o_m[:], wt[:, kk, i], at8[kk][:, i, :],
                                start=(kk == 0 and i == 0),
                                stop=(kk == KT2 - 1 and i == 1))
                ot = ain.tile([128, 2 * TPC], BF16, tag="ot", name="ot", bufs=2)
                if OSC == 1.0:
                    if m % 2 == 0:
                        nc.scalar.copy(ot[:], o_m[:])
                    else:
                        nc.vector.tensor_copy(ot[:], o_m[:])
                else:
                    nc.scalar.mul(ot[:], o_m[:], OSC)
                nc.scalar.dma_start(outT[ts(m, 128), :], ot[:])

        if phases in ("qkv", "norope"):
            # DCE-proof: flush every phase-1 product to outT
            for m in range(H_PER_CORE):
                nc.sync.dma_start(outT[ts(m, 128), :], qt_sb[m][:, ts(rep % NB, 512)])
            nc.sync.dma_start(outT[ts(4, 128), :], kt_sb[:, ts(rep % NB, 512)])
            nc.sync.dma_start(outT[ts(5, 128), :], vt_sb[:, ts(rep % NB, 512)])
            nc.sync.dma_start(outT[ts(6, 128), 0:128], va_sb[:, 0, :])
            return
        attention_batch(0)
        if phases == "attn":
            attention_batch(1)
            nc.sync.dma_start(outT[0:128, :], kt_sb[:, 0:512])
            return
        a2a_batch(0)
        at8_loads(0)
        attention_batch(1)
        a2a_batch(1)
        at8_loads(1)
        wo_all()


def prepare_inputs(x, cos, sin, wq, wk, wv, wo):
    """Host-side: slice/transpose/cast all per-core arrays."""
    import ml_dtypes
    s4 = float(HEAD_DIM) ** -0.25

    PNP = ml_dtypes.float8_e4m3 if PROJ_FP8 else ml_dtypes.bfloat16
    WNP = ml_dtypes.float8_e4m3 if WO_FP8 else ml_dtypes.bfloat16

    # x^T in [KT2, NB, 128, 2, 512]: each (kk, block) tile is one fully
    # contiguous 256 KB DMA (2 KB+ per partition line for full DMA speed)
    xr = (np.asarray(x, np.float32).reshape(NB, 512, KT2, 2, 128) * XSI)
    xT = np.ascontiguousarray(xr.transpose(2, 0, 4, 3, 1)).astype(PNP)

    cosT = (np.ascontiguousarray(cos.T) * (s4 / PF)).astype(ml_dtypes.bfloat16)
    sinT = (np.ascontiguousarray(sin.T) * (s4 / PF)).astype(ml_dtypes.bfloat16)

    # rotate-half matrix: (P @ u) = [-u2; u1];  lhsT = P^T
    P = np.zeros((128, 128), np.float32)
    for d in range(64):
        P[d, d + 64] = -1.0
        P[d + 64, d] = 1.0
    PT = P.T.copy()

    ones = np.ones((128, 128), np.float32)

    # diagonal-block mask for scores^T [k, q]: valid iff k <= q
    kk = np.arange(128)[:, None]
    qq = np.arange(128)[None, :]
    tri = (kk <= qq).astype(np.float32)

    def wslices(w, rows_per_core):
        # w: [out, DIM] -> per-core [128, KT2, 2, rows] (DoubleRow lhsT tiles)
        out = []
        for c in range(N_CORES):
            wc = w[c * rows_per_core:(c + 1) * rows_per_core, :] * WSI
            wt = wc.reshape(rows_per_core, KT2, 2, 128).transpose(3, 1, 2, 0)
            out.append(np.ascontiguousarray(wt).reshape(128, -1).astype(PNP))
        return out

    wq_c = wslices(wq, DQ)
    wk_c = wslices(wk, 128)
    wv_c = wslices(wv, 128)

    # wo streamed: [32 m, 128 p, KT2, 2, 128 j]; every core gets the full wo
    wos = (np.asarray(wo, np.float32) * WOSI).reshape(32, 128, KT2, 2, 128)
    wos = np.ascontiguousarray(wos.transpose(0, 4, 2, 3, 1)).reshape(32, 128, -1)
    wos = wos.astype(WNP)

    bf = lambda a: np.asarray(a, np.float32).astype(ml_dtypes.bfloat16)

    in_maps = []
    for c in range(N_CORES):
        in_maps.append({
            "xT": xT,
            "wq": wq_c[c],
            "wk": wk_c[c],
            "wv": wv_c[c],
            "wos": wos,
            "cosT": cosT,
            "sinT": sinT,
            "pmat": bf(PT),
            "tri": bf(tri),
            "ident": bf(np.eye(128, dtype=np.float32)),
            "ones": bf(ones),
        })
    return in_maps


_cached = {}


def _get_program():
    if "nc" not in _cached:
        _cached["nc"] = build_program()
    return _cached["nc"]


def kernel(x, cos, sin, wq, wk, wv, wo, start_pos):
    assert int(start_pos) == 0
    nc = _get_program()
    in_maps = prepare_inputs(np.asarray(x, np.float32), np.asarray(cos, np.float32),
                             np.asarray(sin, np.float32), np.asarray(wq, np.float32),
                             np.asarray(wk, np.float32), np.asarray(wv, np.float32),
                             np.asarray(wo, np.float32))
    res = run_bass_kernel_spmd(nc, in_maps, core_ids=list(range(N_CORES)))
    # outT per core: [4096 out, 2*TPC tok]; core c owns tokens
    # [b, TPC*c : TPC*c+TPC) for each batch b.
    out = np.empty((BATCH, SEQ, DIM), np.float32)
    for c in range(N_CORES):
        oc = np.asarray(res.results[c]["outT"], np.float32)
        for b in range(BATCH):
            out[b, TPC * c:TPC * (c + 1), :] = oc[:, b * TPC:(b + 1) * TPC].T
    return np.ascontiguousarray(out, dtype=np.float32)


# revision 29
# speedup vs baseline: 29.9743x; 29.9743x over previous
"""Trainium2 Bass kernel for multi-head GQA attention (dense transformer layer).

Problem: x[2,2048,4096] -> attention(RoPE, GQA 32q/8kv heads, causal) -> out[2,2048,4096]

Strategy (8 NeuronCores):
  - QKV tensor-parallel by heads: core c owns q-heads 4c..4c+3 and kv-head c.
    Activations kept feature-on-partition: X^T [din, tok], Q^T/K^T [d, tok],
    scores^T [k, q] -- softmax denominators are ones-matmuls, no transposes.
  - Attention outputs are resharded head->token with a per-batch AllToAll
    (2 MB per core instead of a 16 MB-out AllGather, ~9 us on the wire); each
    core then computes the output projection for its 512 tokens (256 per
    batch) in one N=512 sweep against the FULL wo matrix, streamed from DRAM
    (weights have no reuse there anyway).
  - gpsimd queue carries ONLY the collectives; the softmax-denominator
    broadcast is a tiny ones-matmul on PE, so the AllToAll never blocks the
    attention pipeline of the next batch.
  - Matmuls in bf16 with fp32 PSUM accumulation. fp8 DoubleRow paths exist
    behind PROJ_FP8/WO_FP8 but are OFF: for zero-mean dot products the
    relative error of e4m3 inputs does NOT average down with contraction
    length (signal and noise both grow as sqrt(K)), measured ~8e-2 rel err.
"""

import numpy as np
from contextlib import ExitStack

import concourse.bass as bass
import concourse.tile as tile
from concourse import bacc, mybir
from concourse.bass import ts
from concourse.bass_utils import run_bass_kernel_spmd

BF16 = mybir.dt.bfloat16
F32 = mybir.dt.float32
FP8 = mybir.dt.float8e4

N_CORES = 8
DIM = 4096
N_HEADS = 32
HEAD_DIM = 128
BATCH = 2
SEQ = 2048

TOK = BATCH * SEQ            # 4096 tokens, batch-major
NB = TOK // 512              # 8 token blocks of 512
KT2 = DIM // 256             # 16 double-contraction tiles for projections
H_PER_CORE = N_HEADS // N_CORES       # 4
DQ = H_PER_CORE * HEAD_DIM            # 512 q-dims per core
QB = SEQ // 512              # 4 query blocks of 512 per batch
SKT = SEQ // 128             # 16 key tiles of 128 per batch
TPC = SEQ // N_CORES         # 256 tokens owned per core per batch

# ---- dtype configuration (fp8 flips these) ----
PROJ_FP8 = False             # QKV projection inputs/weights in fp8 e4m3
WO_FP8 = False               # wo weights + attention outputs in fp8 e4m3
PDT = FP8 if PROJ_FP8 else BF16
WDT = FP8 if WO_FP8 else BF16
XSI = 16.0 if PROJ_FP8 else 1.0      # x pre-scale (host)
WSI = 256.0 if PROJ_FP8 else 1.0     # wq/wk/wv pre-scale (host)
PF = XSI * WSI                        # QKV psum excess factor
ATSI = 16.0 if WO_FP8 else 1.0       # attention-output pre-scale (device)
WOSI = 256.0 if WO_FP8 else 1.0      # wo pre-scale (host)
OSC = 1.0 / (ATSI * WOSI)            # final output correction

DR = mybir.MatmulPerfMode.DoubleRow

EXP = mybir.ActivationFunctionType.Exp


def build_program(reps: int = 1, n_cores: int = N_CORES,
                  collective: bool = True, phases: str = "full") -> bass.Bass:
    nc = bacc.Bacc("TRN2", target_bir_lowering=False, debug=False,
                   num_devices=n_cores)

    # ---- I/O (per-core tensors; host pre-arranges layouts) ----
    xT = nc.dram_tensor("xT", [KT2, NB, 128, 2, 512], PDT, kind="ExternalInput").ap()
    wq = nc.dram_tensor("wq", [128, KT2 * 2 * DQ], PDT, kind="ExternalInput").ap()
    wk = nc.dram_tensor("wk", [128, KT2 * 2 * 128], PDT, kind="ExternalInput").ap()
    wv = nc.dram_tensor("wv", [128, KT2 * 2 * 128], PDT, kind="ExternalInput").ap()
    wos = nc.dram_tensor("wos", [32, 128, KT2 * 2 * 128], WDT,
                         kind="ExternalInput").ap()
    cosT = nc.dram_tensor("cosT", [128, SEQ], BF16, kind="ExternalInput").ap()
    sinT = nc.dram_tensor("sinT", [128, SEQ], BF16, kind="ExternalInput").ap()
    pmat = nc.dram_tensor("pmat", [128, 128], BF16, kind="ExternalInput").ap()
    tri = nc.dram_tensor("tri", [128, 128], BF16, kind="ExternalInput").ap()
    ident = nc.dram_tensor("ident", [128, 128], BF16, kind="ExternalInput").ap()
    ones = nc.dram_tensor("ones", [128, 128], BF16, kind="ExternalInput").ap()
    outT = nc.dram_tensor("outT", [DIM, BATCH * TPC], BF16, kind="ExternalOutput").ap()

    # internal DRAM for the collectives (cannot use I/O tensors); one pair
    # per (rep, batch) so reps never race and batches can overlap.
    cc_in = [[nc.dram_tensor(f"cc_in{r}_{b}", [N_CORES, DQ, TPC], WDT)
              for b in range(BATCH)] for r in range(reps)]
    cc_out = [[nc.dram_tensor(f"cc_out{r}_{b}", [N_CORES, DQ, TPC], WDT)
               for b in range(BATCH)] for r in range(reps)]

    with tile.TileContext(nc) as tc, ExitStack() as top:
        consts = top.enter_context(tc.tile_pool(name="consts", bufs=1))
        weights = top.enter_context(tc.tile_pool(name="weights", bufs=1))
        acts = top.enter_context(tc.tile_pool(name="acts", bufs=1))

        # weights: the very first matmuls gate only on the kk=0..3 chunks;
        # everything else is emitted right after the first x-tile DMA (see
        # the late_loads callback below).
        wq_sb = weights.tile([128, KT2, 2, DQ], PDT)
        wk_sb = weights.tile([128, KT2, 2, 128], PDT)
        wv_sb = weights.tile([128, KT2, 2, 128], PDT)
        wq4 = wq.rearrange("p (k i m) -> p k i m", k=KT2, i=2)
        wk4 = wk.rearrange("p (k i m) -> p k i m", k=KT2, i=2)
        wv4 = wv.rearrange("p (k i m) -> p k i m", k=KT2, i=2)
        ck = 2
        for kki in range(ck):
            k0 = slice(kki, kki + 1)
            nc.sync.dma_start(wq_sb[:, k0], wq4[:, k0])
            nc.sync.dma_start(wk_sb[:, k0], wk4[:, k0])
            nc.sync.dma_start(wv_sb[:, k0], wv4[:, k0])

        # tiny constants next
        pm_sb = consts.tile([128, 128], BF16)
        nc.sync.dma_start(pm_sb[:], pmat[:, :])
        tri_sb = consts.tile([128, 128], BF16)
        nc.sync.dma_start(tri_sb[:], tri[:, :])
        id_sb = consts.tile([128, 128], BF16)
        nc.sync.dma_start(id_sb[:], ident[:, :])
        ones_sb = consts.tile([128, 128], BF16)
        nc.sync.dma_start(ones_sb[:], ones[:, :])
        atsi_sb = consts.tile([1, 128], BF16)
        nc.vector.memset(atsi_sb[:], ATSI)

        cos_sb = consts.tile([128, SEQ], BF16)
        sin_sb = consts.tile([128, SEQ], BF16)

        def late_loads():
            ksl = slice(ck, KT2)  # ck == 2
            nc.sync.dma_start(wq_sb[:, ksl], wq4[:, ksl])
            nc.sync.dma_start(wk_sb[:, ksl], wk4[:, ksl])
            nc.sync.dma_start(wv_sb[:, ksl], wv4[:, ksl])
            nc.sync.dma_start(cos_sb[:], cosT[:, :])
            nc.sync.dma_start(sin_sb[:], sinT[:, :])
        run_body.late_loads = late_loads

        # per-core activations (feature-major)
        qt_sb = [acts.tile([128, TOK], BF16, tag=f"qt{m}", name=f"qt{m}")
                 for m in range(H_PER_CORE)]
        kt_sb = acts.tile([128, TOK], BF16)
        vt_sb = acts.tile([128, TOK], BF16)          # V^T, pre-transpose
        va_sb = acts.tile([128, 2 * KT2, 128], BF16)  # V in [tok, dv] tiles

        run_body.wos_dram = wos
        for rep in range(reps):
            run_body(nc, tc, rep, cc_in[rep], cc_out[rep], outT,
                     wq_sb, wk_sb, wv_sb, cos_sb, sin_sb, pm_sb,
                     tri_sb, id_sb, ones_sb, atsi_sb, qt_sb, kt_sb, vt_sb,
                     va_sb, xT, wos, n_cores=n_cores, collective=collective,
                     phases=phases)

    nc.compile()
    return nc


def run_body(nc, tc, rep, cc_in, cc_out, outT,
             wq_sb, wk_sb, wv_sb, cos_sb, sin_sb, pm_sb,
             tri_sb, id_sb, ones_sb, atsi_sb, qt_sb, kt_sb, vt_sb, va_sb, xT,
             wos, n_cores=N_CORES, collective=True, phases="full"):
    # ---------------- phase 1: QKV projection + RoPE ----------------
    with ExitStack() as body:
      ps = body.enter_context(tc.tile_pool(name=f"ps_{rep}", bufs=1, space="PSUM"))
      with ExitStack() as ph:
        xin = ph.enter_context(tc.tile_pool(name=f"xin{rep}", bufs=4))
        rope = ph.enter_context(tc.tile_pool(name=f"rope{rep}", bufs=2))

        for n in range(NB):
            s0 = n % QB                 # 512-block position within the batch
            q_ps = [ps.tile([128, 512], F32, tag=f"qps{m}", bufs=1, name=f"qps{m}")
                    for m in range(H_PER_CORE)]
            k_ps = ps.tile([128, 512], F32, tag="kps", bufs=1)
            v_ps = ps.tile([128, 512], F32, tag="vps", bufs=1)
            for kk in range(KT2):
                xt = xin.tile([128, 2, 512], PDT, tag="xt")
                nc.sync.dma_start(xt[:], xT[kk, n])
                if rep == 0 and n == 0 and kk == 2 and run_body.late_loads:
                    run_body.late_loads()
                    run_body.late_loads = None
                st, sp = (kk == 0), (kk == KT2 - 1)
                if PROJ_FP8:
                    for m in range(H_PER_CORE):
                        nc.tensor.matmul(q_ps[m][:], wq_sb[:, kk, :, ts(m, 128)],
                                         xt[:], start=st, stop=sp, perf_mode=DR)
                    nc.tensor.matmul(k_ps[:], wk_sb[:, kk], xt[:],
                                     start=st, stop=sp, perf_mode=DR)
                    nc.tensor.matmul(v_ps[:], wv_sb[:, kk], xt[:],
                                     start=st, stop=sp, perf_mode=DR)
                else:
                    for i in range(2):
                        sti, spi = st and i == 0, sp and i == 1
                        for m in range(H_PER_CORE):
                            nc.tensor.matmul(q_ps[m][:], wq_sb[:, kk, i, ts(m, 128)],
                                             xt[:, i, :], start=sti, stop=spi)
                        nc.tensor.matmul(k_ps[:], wk_sb[:, kk, i], xt[:, i, :],
                                         start=sti, stop=spi)
                        nc.tensor.matmul(v_ps[:], wv_sb[:, kk, i], xt[:, i, :],
                                         start=sti, stop=spi)

            # V^T: copy out of PSUM (scale folds away the fp8 pre-scales)
            if PF == 1.0:
                nc.scalar.copy(vt_sb[:, ts(n, 512)], v_ps[:])
            else:
                nc.scalar.mul(vt_sb[:, ts(n, 512)], v_ps[:], 1.0 / PF)

            # RoPE on Q heads and K:  y = raw*cos + (P@raw)*sin
            # (softmax scale and 1/PF folded into the cos/sin tables)
            def do_rope(acc, dst, eng):
                raw = rope.tile([128, 512], BF16, tag="raw")
                eng.tensor_copy(raw[:], acc[:]) if eng is nc.vector \
                    else eng.copy(raw[:], acc[:])
                rot = ps.tile([128, 512], F32, tag="rot", bufs=2)
                nc.tensor.matmul(rot[:], pm_sb[:], raw[:], start=True, stop=True)
                t1 = rope.tile([128, 512], BF16, tag="t1")
                nc.vector.tensor_mul(t1[:], raw[:], cos_sb[:, ts(s0, 512)])
                t2 = rope.tile([128, 512], BF16, tag="t2")
                nc.vector.tensor_mul(t2[:], rot[:], sin_sb[:, ts(s0, 512)])
                nc.vector.tensor_add(dst, t1[:], t2[:])

            if phases == "norope":
                for m in range(H_PER_CORE):
                    eng = nc.scalar if m % 2 == 0 else nc.vector
                    if eng is nc.vector:
                        eng.tensor_copy(qt_sb[m][:, ts(n, 512)], q_ps[m][:])
                    else:
                        eng.copy(qt_sb[m][:, ts(n, 512)], q_ps[m][:])
                nc.scalar.copy(kt_sb[:, ts(n, 512)], k_ps[:])
            else:
                for m in range(H_PER_CORE):
                    do_rope(q_ps[m], qt_sb[m][:, ts(n, 512)],
                            nc.scalar if m % 2 == 0 else nc.vector)
                do_rope(k_ps, kt_sb[:, ts(n, 512)], nc.scalar)

            # V^T -> V transposes for this block (shares the rot psum slots)
            for t in range(4 * n, 4 * n + 4):
                tr = ps.tile([128, 128], BF16, tag="rot", bufs=2, name="tr")
                nc.tensor.transpose(tr[:], vt_sb[:, ts(t, 128)], id_sb[:])
                nc.vector.tensor_copy(va_sb[:, t, :], tr[:])

      # ------- phase 2: attention; a2a per batch overlaps next phase -------
      with ExitStack() as ph:
        work = ph.enter_context(tc.tile_pool(name=f"attnwork{rep}", bufs=4))

        def attention_batch(b):
            for qb in range(QB):
                gq = b * SEQ + qb * 512
                nkt = (qb + 1) * 4
                atags = ["qps3", "kps", "vps", "rot"]
                o_ps = [ps.tile([128, 512], F32, tag=atags[h],
                                bufs=(2 if atags[h] == "rot" else 1),
                                name=f"aops{h}") for h in range(H_PER_CORE)]
                # one psum bank holds all 4 heads' softmax denominators at
                # 32-aligned partition offsets (col-group packed matmuls)
                dn_ps = ps.tile([128, 512], F32, tag="rot", bufs=2, name="dn")
                ex_prev = [None] * H_PER_CORE
                off_prev = [0] * H_PER_CORE
                for kt in range(nkt):
                    gk = b * SEQ + kt * 128
                    vtile = b * SKT + kt
                    j = kt - qb * 4          # >= 0 -> diagonal tile
                    q_off = 128 * j if j > 0 else 0
                    N = 512 - q_off
                    st, sp = (kt == 0), (kt == nkt - 1)
                    # score matmuls run ahead of the exp+PV loop: the PE works
                    # through the scores while the scalar engine exps the
                    # previous heads, instead of stalling per head. Only 3
                    # psum banks rotate, so head 3's score matmul must be
                    # emitted after head 0's exp (which frees bank 0).
                    s_ps, exs = [], []

                    def emit_s(h):
                        sp_t = ps.tile([128, 512], F32,
                                       tag=f"qps{(kt * 4 + h) % 3}", bufs=1,
                                       name="sps")
                        nc.tensor.matmul(sp_t[:, :N], kt_sb[:, gk:gk + 128],
                                         qt_sb[h][:, gq + q_off:gq + 512],
                                         start=True, stop=True)
                        s_ps.append(sp_t)

                    def emit_exp(h):
                        ex = work.tile([128, 512], BF16, tag="expT", bufs=12,
                                       name="ex")
                        nc.scalar.activation(ex[:, :N], s_ps[h][:, :N], EXP)
                        exs.append(ex)

                    for h in range(3):
                        emit_s(h)
                    emit_exp(0)
                    emit_s(3)
                    for h in range(1, 4):
                        emit_exp(h)
                    for h in range(H_PER_CORE):
                        ex = exs[h]
                        if j >= 0:
                            nc.vector.tensor_mul(ex[:, :128], ex[:, :128],
                                                 tri_sb[:])
                        nc.tensor.matmul(o_ps[h][:, q_off:], va_sb[:, vtile, :],
                                         ex[:, :N], start=st, stop=sp)
                        if kt % 2 == 0:
                            ex_prev[h] = ex
                            off_prev[h] = q_off
                        else:
                            # fold this tile's exp sums into the previous
                            # tile (bf16 add), one denominator matmul per pair
                            exp_, offp = ex_prev[h], off_prev[h]
                            d = q_off - offp
                            nc.vector.tensor_add(exp_[:, d:512 - offp],
                                                 exp_[:, d:512 - offp],
                                                 ex[:, :N])
                            nc.tensor.matmul(
                                dn_ps[32 * h:32 * h + 1, offp:],
                                ones_sb[:, 0:1], exp_[:, :512 - offp],
                                start=(kt == 1), stop=(kt == nkt - 1),
                                tile_position=(0, 32 * h))
                            ex_prev[h] = None
                # normalization chain, grouped per step so the psum "rot"
                # buffers reused by bc are provably drained (all dn/o readers
                # emitted first) and PE never WARs onto the lagging DVE chain.
                recs, rec16s = [], []
                for h in range(H_PER_CORE):
                    rec = work.tile([1, 512], F32, tag="rec", name="rec")
                    nc.vector.reciprocal(rec[:], dn_ps[32 * h:32 * h + 1, :])
                    recs.append(rec)
                for h in range(H_PER_CORE):
                    rec16 = work.tile([1, 512], BF16, tag="rec16", name="rec16")
                    nc.vector.tensor_copy(rec16[:], recs[h][:])
                    rec16s.append(rec16)
                ocs = []
                for h in range(H_PER_CORE):
                    oc = work.tile([128, 512], F32, tag="oc", name="oc")
                    nc.vector.tensor_copy(oc[:], o_ps[h][:])   # frees psum bank
                    ocs.append(oc)
                for h in range(H_PER_CORE):
                    # broadcast ATSI/denominator across partitions via PE
                    bc = ps.tile([128, 512], F32, tag="rot", bufs=2, name="bc")
                    nc.tensor.matmul(bc[:], atsi_sb[:], rec16s[h][:],
                                     start=True, stop=True)
                    at = work.tile([128, 512], WDT, tag="at", name="at")
                    nc.vector.tensor_mul(at[:], ocs[h][:], bc[:])
                    # scatter the two 256-token halves to their a2a chunks
                    # (two partition-major DMAs; a fused chunk-leading AP is
                    # not a legal SBUF access pattern)
                    for i in range(2):
                        nc.sync.dma_start(
                            cc_in[b].ap()[2 * qb + i,
                                          h * 128:(h + 1) * 128, :],
                            at[:, i * TPC:(i + 1) * TPC])

        def a2a_batch(b):
            if collective:
                nc.gpsimd.collective_compute(
                    "AllToAll",
                    mybir.AluOpType.bypass,
                    ins=[cc_in[b].ap().opt()],
                    outs=[cc_out[b].ap().opt()],
                    replica_groups=[list(range(n_cores))],
                )

        ain = ph.enter_context(tc.tile_pool(name=f"ain{rep}", bufs=2))
        wos3 = wos  # [32, 128, KT2*2*128]
        at8 = []

        def at8_loads(b):
            # this core owns tokens [b, TPC*core .. TPC*core+TPC); the a2a
            # output is exactly attn^T [4096 hd, TPC] for those tokens.
            # at8 loads are triggered from the gpsimd queue, IN ORDER AFTER
            # the AllToAll: their descriptors only reach the DMA engines once
            # the collective is done, so they never park at a queue head
            # blocking unrelated transfers. Batch b fills columns
            # [b*TPC, b*TPC+TPC) so the wo matmuls sweep both batches at once.
            at_flat = cc_out[b].ap().rearrange("c r t -> (c r) t")
            for kk in range(KT2):
                if b == 0:
                    at8.append(ain.tile([128, 2, 2 * TPC], WDT,
                                        tag=f"at8_{kk}", bufs=1,
                                        name=f"at8_{kk}"))
                t8 = at8[kk]
                for i in range(2):
                    nc.gpsimd.dma_start(t8[:, i, ts(b, TPC)],
                                        at_flat[256 * kk + 128 * i:
                                                256 * kk + 128 * i + 128, :])

        def wo_all():
            wtags = ["qps3", "kps", "vps", "rot"]
            for m in range(32):
                wt = ain.tile([128, KT2, 2, 128], WDT, tag="wot", bufs=3,
                              name="wot")
                nc.sync.dma_start(wt[:], wos3[m].rearrange(
                    "p (k i m) -> p k i m", k=KT2, i=2))
                o_m = ps.tile([128, 512], F32, tag=wtags[m % 4],
                              bufs=(2 if wtags[m % 4] == "rot" else 1),
                              name=f"wops{m % 4}")
                if WO_FP8:
                    for kk in range(KT2):
                        nc.tensor.matmul(o_m[:], wt[:, kk], at8[kk][:],
                                         start=(kk == 0), stop=(kk == KT2 - 1),
                                         perf_mode=DR)
                else:
                    for kk in range(KT2):
                        for i in range(2):
                            nc.tensor.matmul(
                                o_m[:], wt[:, kk, i], at8[kk][:, i, :],
                                start=(kk == 0 and i == 0),
                                stop=(kk == KT2 - 1 and i == 1))
                ot = ain.tile([128, 2 * TPC], BF16, tag="ot", name="ot", bufs=2)
                if OSC == 1.0:
                    if m % 2 == 0:
                        nc.scalar.copy(ot[:], o_m[:])
                    else:
                        nc.vector.tensor_copy(ot[:], o_m[:])
                else:
                    nc.scalar.mul(ot[:], o_m[:], OSC)
                nc.scalar.dma_start(outT[ts(m, 128), :], ot[:])

        if phases in ("qkv", "norope"):
            # DCE-proof: flush every phase-1 product to outT
            for m in range(H_PER_CORE):
                nc.sync.dma_start(outT[ts(m, 128), :], qt_sb[m][:, ts(rep % NB, 512)])
            nc.sync.dma_start(outT[ts(4, 128), :], kt_sb[:, ts(rep % NB, 512)])
            nc.sync.dma_start(outT[ts(5, 128), :], vt_sb[:, ts(rep % NB, 512)])
            nc.sync.dma_start(outT[ts(6, 128), 0:128], va_sb[:, 0, :])
            return
        attention_batch(0)
        if phases == "attn":
            attention_batch(1)
            nc.sync.dma_start(outT[0:128, :], kt_sb[:, 0:512])
            return
        a2a_batch(0)
        at8_loads(0)
        attention_batch(1)
        a2a_batch(1)
        at8_loads(1)
        wo_all()


def prepare_inputs(x, cos, sin, wq, wk, wv, wo):
    """Host-side: slice/transpose/cast all per-core arrays."""
    import ml_dtypes
    s4 = float(HEAD_DIM) ** -0.25

    PNP = ml_dtypes.float8_e4m3 if PROJ_FP8 else ml_dtypes.bfloat16
    WNP = ml_dtypes.float8_e4m3 if WO_FP8 else ml_dtypes.bfloat16

    # x^T in [KT2, NB, 128, 2, 512]: each (kk, block) tile is one fully
    # contiguous 256 KB DMA (2 KB+ per partition line for full DMA speed)
    xr = (np.asarray(x, np.float32).reshape(NB, 512, KT2, 2, 128) * XSI)
    xT = np.ascontiguousarray(xr.transpose(2, 0, 4, 3, 1)).astype(PNP)

    cosT = (np.ascontiguousarray(cos.T) * (s4 / PF)).astype(ml_dtypes.bfloat16)
    sinT = (np.ascontiguousarray(sin.T) * (s4 / PF)).astype(ml_dtypes.bfloat16)

    # rotate-half matrix: (P @ u) = [-u2; u1];  lhsT = P^T
    P = np.zeros((128, 128), np.float32)
    for d in range(64):
        P[d, d + 64] = -1.0
        P[d + 64, d] = 1.0
    PT = P.T.copy()

    ones = np.ones((128, 128), np.float32)

    # diagonal-block mask for scores^T [k, q]: valid iff k <= q
    kk = np.arange(128)[:, None]
    qq = np.arange(128)[None, :]
    tri = (kk <= qq).astype(np.float32)

    def wslices(w, rows_per_core):
        # w: [out, DIM] -> per-core [128, KT2, 2, rows] (DoubleRow lhsT tiles)
        out = []
        for c in range(N_CORES):
            wc = w[c * rows_per_core:(c + 1) * rows_per_core, :] * WSI
            wt = wc.reshape(rows_per_core, KT2, 2, 128).transpose(3, 1, 2, 0)
            out.append(np.ascontiguousarray(wt).reshape(128, -1).astype(PNP))
        return out

    wq_c = wslices(wq, DQ)
    wk_c = wslices(wk, 128)
    wv_c = wslices(wv, 128)

    # wo streamed: [32 m, 128 p, KT2, 2, 128 j]; every core gets the full wo
    wos = (np.asarray(wo, np.float32) * WOSI).reshape(32, 128, KT2, 2, 128)
    wos = np.ascontiguousarray(wos.transpose(0, 4, 2, 3, 1)).reshape(32, 128, -1)
    wos = wos.astype(WNP)

    bf = lambda a: np.asarray(a, np.float32).astype(ml_dtypes.bfloat16)

    in_maps = []
    for c in range(N_CORES):
        in_maps.append({
            "xT": xT,
            "wq": wq_c[c],
            "wk": wk_c[c],
            "wv": wv_c[c],
            "wos": wos,
            "cosT": cosT,
            "sinT": sinT,
            "pmat": bf(PT),
            "tri": bf(tri),
            "ident": bf(np.eye(128, dtype=np.float32)),
            "ones": bf(ones),
        })
    return in_maps


_cached = {}


def _get_program():
    if "nc" not in _cached:
        _cached["nc"] = build_program()
    return _cached["nc"]


def kernel(x, cos, sin, wq, wk, wv, wo, start_pos):
    assert int(start_pos) == 0
    nc = _get_program()
    in_maps = prepare_inputs(np.asarray(x, np.float32), np.asarray(cos, np.float32),
                             np.asarray(sin, np.float32), np.asarray(wq, np.float32),
                             np.asarray(wk, np.float32), np.asarray(wv, np.float32),
                             np.asarray(wo, np.float32))
    res = run_bass_kernel_spmd(nc, in_maps, core_ids=list(range(N_CORES)))
    # outT per core: [4096 out, 2*TPC tok]; core c owns tokens
    # [b, TPC*c : TPC*c+TPC) for each batch b.
    out = np.empty((BATCH, SEQ, DIM), np.float32)
    for c in range(N_CORES):
        oc = np.asarray(res.results[c]["outT"], np.float32)
        for b in range(BATCH):
            out[b, TPC * c:TPC * (c + 1), :] = oc[:, b * TPC:(b + 1) * TPC].T
    return np.ascontiguousarray(out, dtype=np.float32)
